# revision 1
# baseline (speedup 1.0000x reference)
"""Trainium2 Bass kernel for CrossAttentionFusion.

Reference computation (shapes hardcoded):
  B=4, C=256, H=W=128, N=16384, CHUNK=2048, nchunks=8.
  q  = image_features  reshaped to (B, nchunks, CHUNK, C)
  kv = lidar_features  reshaped to (B, nchunks, CHUNK, C)
  per (b, chunk): out = softmax(q @ kv.T / sqrt(C)) @ kv
  output = w0 * image_features + w1 * fused,  w = softmax(modality_weights)

Sharding: the 32 independent (b, chunk) pairs are split 4-per-core across
8 NeuronCores (data parallel over batch x chunk; no communication).

Per-core device kernel, per (b, chunk) pair (all layouts partition-major):
  DRAM holds Q^T / KV^T naturally as (C, CHUNK) slices.
  1. DMA Q^T, KV^T (fp32), cast to bf16 (DVE).
  2. PE-transpose KV^T -> KV (k, c) tiles with an appended ones column.
  3. mm1: S^T tile (k=128p, q=512f) = KVt.T @ Qt  (bf16, fp32 PSUM accum).
  4. ACT exp with scale 1/sqrt(C): P^T panel in SBUF (bf16).
  5. mm2: (q=128p, 257f) = P^T.T @ [KV | 1]  -> unnormalized out | rowsum.
  6. DVE: G = (O' * recip(rowsum)) * w1 (bf16), PE-transpose G -> (c, q),
     DVE: out = (Q^T * w0) + G^T (fp32), DMA out.
"""

import numpy as np

B, C, H, W = 4, 256, 128, 128
N = H * W
CHUNK = 2048
NCHUNKS = N // CHUNK         # 8
NCORES = 8
PAIRS = B * NCHUNKS          # 32
PPC = PAIRS // NCORES        # 4 pairs (chunks) per core
CT = C // 128                # 2 c-tiles
KT = CHUNK // 128            # 16 k-tiles
PAN = 512                    # q panel width
NPAN = CHUNK // PAN          # 4 panels
QT_PER_PAN = PAN // 128      # 4 q-tiles per panel
SCALE = 1.0 / float(np.sqrt(C))

_BUILD_CACHE = {}


def _build(w0: float, w1: float):
    from contextlib import ExitStack

    import concourse.bass as bass
    import concourse.tile as tile
    from concourse import bacc, masks, mybir

    f32 = mybir.dt.float32
    bf16 = mybir.dt.bfloat16
    Exp = mybir.ActivationFunctionType.Exp
    mult = mybir.AluOpType.mult
    add = mybir.AluOpType.add

    nc = bacc.Bacc("TRN2", target_bir_lowering=False, debug=False)
    q_d = nc.dram_tensor("q_sh", (PPC, C, CHUNK), f32, kind="ExternalInput")
    kv_d = nc.dram_tensor("kv_sh", (PPC, C, CHUNK), f32, kind="ExternalInput")
    out_d = nc.dram_tensor("out_sh", (PPC, C, CHUNK), f32, kind="ExternalOutput")

    with ExitStack() as ctx:
        tc = ctx.enter_context(tile.TileContext(nc))
        po_const = ctx.enter_context(tc.tile_pool(name="const", bufs=1))
        po_qf = ctx.enter_context(tc.tile_pool(name="qf", bufs=2))
        po_kvf = ctx.enter_context(tc.tile_pool(name="kvf", bufs=2))
        po_qb = ctx.enter_context(tc.tile_pool(name="qb", bufs=2))
        po_kvb = ctx.enter_context(tc.tile_pool(name="kvb", bufs=2))
        po_kc = ctx.enter_context(tc.tile_pool(name="kc", bufs=2))
        po_pt = ctx.enter_context(tc.tile_pool(name="pt", bufs=2))
        po_out = ctx.enter_context(tc.tile_pool(name="outs", bufs=2))
        po_g = ctx.enter_context(tc.tile_pool(name="g", bufs=2))
        po_r = ctx.enter_context(tc.tile_pool(name="r", bufs=2))
        po_psS = ctx.enter_context(tc.tile_pool(name="psS", bufs=2, space="PSUM"))
        po_psO = ctx.enter_context(tc.tile_pool(name="psO", bufs=2, space="PSUM"))
        po_psT = ctx.enter_context(tc.tile_pool(name="psT", bufs=4, space="PSUM"))

        ident = po_const.tile([128, 128], bf16, name="ident")
        masks.make_identity(nc, ident[:])

        for p in range(PPC):
            qf = po_qf.tile([128, CT * CHUNK], f32, name="qf")
            kvf = po_kvf.tile([128, CT * CHUNK], f32, name="kvf")
            for ci in range(CT):
                nc.sync.dma_start(
                    qf[:, ci * CHUNK : (ci + 1) * CHUNK],
                    q_d[p, ci * 128 : (ci + 1) * 128, :],
                )
                nc.sync.dma_start(
                    kvf[:, ci * CHUNK : (ci + 1) * CHUNK],
                    kv_d[p, ci * 128 : (ci + 1) * 128, :],
                )

            qb = po_qb.tile([128, CT * CHUNK], bf16, name="qb")
            nc.vector.tensor_copy(qb[:], qf[:])
            kvb = po_kvb.tile([128, CT * CHUNK], bf16, name="kvb")
            nc.vector.tensor_copy(kvb[:], kvf[:])

            # KV in (k, c) layout, 16 tiles of (128, 257); col 256 stays 1.0
            kc = po_kc.tile([128, KT * 257], bf16, name="kc")
            nc.gpsimd.memset(kc[:], 1.0)
            for j in range(KT):
                for ci in range(CT):
                    pst = po_psT.tile([128, 128], bf16, name="pst")
                    nc.tensor.transpose(
                        pst[:],
                        kvb[:, ci * CHUNK + j * 128 : ci * CHUNK + (j + 1) * 128],
                        ident[:],
                    )
                    eng = nc.vector if ci == 0 else nc.scalar
                    if ci == 0:
                        nc.vector.tensor_copy(
                            kc[:, j * 257 + ci * 128 : j * 257 + (ci + 1) * 128],
                            pst[:],
                        )
                    else:
                        nc.scalar.copy(
                            kc[:, j * 257 + ci * 128 : j * 257 + (ci + 1) * 128],
                            pst[:],
                        )

            outs = po_out.tile([128, CT * CHUNK], f32, name="outs")

            for pan in range(NPAN):
                # mm1 + exp -> P^T panel (k-tile major, 512 q cols each)
                pt = po_pt.tile([128, KT * PAN], bf16, name="pt")
                for j in range(KT):
                    psS = po_psS.tile([128, PAN], f32, name="psS")
                    for ci in range(CT):
                        nc.tensor.matmul(
                            psS[:],
                            lhsT=kvb[
                                :, ci * CHUNK + j * 128 : ci * CHUNK + (j + 1) * 128
                            ],
                            rhs=qb[
                                :, ci * CHUNK + pan * PAN : ci * CHUNK + (pan + 1) * PAN
                            ],
                            start=(ci == 0),
                            stop=(ci == CT - 1),
                        )
                    nc.scalar.activation(
                        pt[:, j * PAN : (j + 1) * PAN], psS[:], Exp, scale=SCALE
                    )

                # mm2 + normalize + transpose + fuse
                for tq in range(QT_PER_PAN):
                    q0 = pan * PAN + tq * 128
                    psO = po_psO.tile([128, C + 1], f32, name="psO")
                    for j in range(KT):
                        nc.tensor.matmul(
                            psO[:],
                            lhsT=pt[:, j * PAN + tq * 128 : j * PAN + (tq + 1) * 128],
                            rhs=kc[:, j * 257 : (j + 1) * 257],
                            start=(j == 0),
                            stop=(j == KT - 1),
                        )
                    r = po_r.tile([128, 1], f32, name="r")
                    nc.vector.reciprocal(r[:], psO[:, C : C + 1])
                    g = po_g.tile([128, C], bf16, name="g")
                    nc.vector.tensor_scalar(
                        g[:], psO[:, 0:C], r[:], float(w1), op0=mult, op1=mult
                    )
                    for ci in range(CT):
                        pgt = po_psT.tile([128, 128], bf16, name="pgt", tag="pst")
                        nc.tensor.transpose(
                            pgt[:], g[:, ci * 128 : (ci + 1) * 128], ident[:]
                        )
                        nc.vector.scalar_tensor_tensor(
                            outs[:, ci * CHUNK + q0 : ci * CHUNK + q0 + 128],
                            qf[:, ci * CHUNK + q0 : ci * CHUNK + q0 + 128],
                            float(w0),
                            pgt[:],
                            op0=mult,
                            op1=add,
                        )

            for ci in range(CT):
                nc.sync.dma_start(
                    out_d[p, ci * 128 : (ci + 1) * 128, :],
                    outs[:, ci * CHUNK : (ci + 1) * CHUNK],
                )

    nc.compile()
    return nc


def _get_nc(w0: float, w1: float):
    key = (round(float(w0), 9), round(float(w1), 9))
    if key not in _BUILD_CACHE:
        _BUILD_CACHE[key] = _build(*key)
    return _BUILD_CACHE[key]


def _shard(arr: np.ndarray) -> list[np.ndarray]:
    # (B, C, H, W) -> (PAIRS, C, CHUNK) -> list of (PPC, C, CHUNK) per core
    pairs = (
        arr.reshape(B, C, NCHUNKS, CHUNK)
        .transpose(0, 2, 1, 3)
        .reshape(PAIRS, C, CHUNK)
    )
    return [
        np.ascontiguousarray(pairs[i * PPC : (i + 1) * PPC], dtype=np.float32)
        for i in range(NCORES)
    ]


def _unshard(per_core: list[np.ndarray]) -> np.ndarray:
    pairs = np.concatenate(per_core, axis=0)  # (PAIRS, C, CHUNK)
    return np.ascontiguousarray(
        pairs.reshape(B, NCHUNKS, C, CHUNK).transpose(0, 2, 1, 3).reshape(B, C, H, W)
    )


def run(lidar_features, image_features, modality_weights, trace=False):
    from concourse import bass_utils

    mw = np.asarray(modality_weights, dtype=np.float64)
    e = np.exp(mw - mw.max())
    wsm = e / e.sum()
    w0, w1 = float(wsm[0]), float(wsm[1])

    nc = _get_nc(w0, w1)

    q_shards = _shard(np.asarray(image_features, dtype=np.float32))
    kv_shards = _shard(np.asarray(lidar_features, dtype=np.float32))
    in_maps = [
        {"q_sh": q_shards[i], "kv_sh": kv_shards[i]} for i in range(NCORES)
    ]
    res = bass_utils.run_bass_kernel_spmd(
        nc, in_maps, core_ids=list(range(NCORES)), trace=trace
    )
    out = _unshard([res.results[i]["out_sh"] for i in range(NCORES)])
    return out, res


def kernel(lidar_features, image_features, modality_weights) -> np.ndarray:
    out, _ = run(lidar_features, image_features, modality_weights, trace=False)
    return out
